# revision 34
# baseline (speedup 1.0000x reference)
"""Trainium2 Bass kernel: causal self-attention with RoPE.

Problem: x[4, 2048, 1024], W_qkv[3072, 1024], W_out[1024, 1024], 16 heads.
Sharding: 8 cores = (batch b, head-group hg of 8 heads); core c -> b=c//2,
hg=c%2. Each core computes a full [S, d_model] partial of the output (its
8 heads' contribution through out_proj); the host sums the two head-group
partials per batch.

On-chip layout is fully "transposed": q^T/k^T are produced as [d, s] tiles
(two heads per 128-partition tile), scores are computed as S^T = [k, q] so
the softmax needs no on-chip transposes, and PV/out_proj consume the
transposed forms directly, producing y in natural [s, e] layout.

RoPE trick: head dims are interleaved host-side (perm 2i<-i, 2i+1<-i+32) so
rotate_half becomes an adjacent-pair swap, which the DVE stream_shuffle can
do (it only permutes within 32-partition quadrants). Signs are folded into
the host-built sin table.

Softmax trick: no max subtraction (logits are ~N(0,1) after the 1/8 scale,
max |logit| < ~7, exp is safe in fp32); denominator comes free from a ones
column appended to V (matmul M=65). Normalization: the two denominator rows
are DMA-staged to partitions {0,1}, a K=2 selector matmul broadcasts them
across 128 partitions, and the reciprocal is computed as ACT exp(-ln(x))
(both functions live in the pinned 'natural_log_exp_and_others' table set,
so there are no ACT table switches). Each q-chunk's out_proj is emitted as
filler chunks between the next chunk's attention kb-steps so the PE never
idles while ACT runs the softmax exps.
"""

import sys
import types
from contextlib import ExitStack

import numpy as np

import concourse.bass as bass
import concourse.mybir as mybir
import concourse.tile as tile
from concourse import bacc, bass_utils

F32 = mybir.dt.float32
F32R = mybir.dt.float32r
AF = mybir.ActivationFunctionType

N_HEADS = 16
ROPE_BASE = 10000.0
B_FULL, S_FULL, DM = 4, 2048, 1024
HPC = 8          # heads per core
D = 64           # head dim
SCALE = 1.0 / 8.0  # D ** -0.5
SC = 512         # s-chunk width
KCN = DM // 128  # 8 contraction chunks for the projections

# matmul input dtype: float32r = full-rate fp32 (tf32-ish precision),
# mybir.dt.float32 = 4x slower exact fp32.
MM_DT = F32R

PAIRSWAP = [i + 1 if i % 2 == 0 else i - 1 for i in range(32)]


def _install_ntff_hook_shim():
    """Register the axon NTFF profiling hook if antenv.axon_hooks is absent."""
    try:
        from antenv import axon_hooks  # noqa: F401
        return
    except ImportError:
        pass
    try:
        import antenv
        from trn_agent_boot.trn_boot import _ntff_profile_via_ctypes
        hook = _ntff_profile_via_ctypes('/opt/axon/libaxon_pjrt.so')
    except Exception:
        return
    mod = types.ModuleType('antenv.axon_hooks')
    mod._hook = hook
    mod.get_axon_ntff_profile_hook = lambda: mod._hook
    mod.set_axon_ntff_profile_hook = lambda h: setattr(mod, '_hook', h)
    sys.modules['antenv.axon_hooks'] = mod
    antenv.axon_hooks = mod


def _pin_act_tables():
    """Force every activation onto 'natural_log_exp_and_others' (it holds
    exp, ln, copy and identity) so the kernel needs exactly one
    ACT_TABLE_LOAD instead of thrashing between the exp and ln sets."""
    import concourse.hw_specs as hw_specs
    if getattr(bacc, '_act_tables_pinned', False):
        return
    orig = hw_specs.get_activation_tables

    def pinned(module_arch):
        tabs = orig(module_arch)
        keep = 'natural_log_exp_and_others'
        if keep in tabs:
            for k in tabs:
                if k != keep:
                    tabs[k] = set()
        return tabs

    bacc.get_activation_tables = pinned
    bacc._act_tables_pinned = True


def build_program(s_len=S_FULL):
    """Build the single-core Bass program (identical across the 8 cores)."""
    _pin_act_tables()
    nc = bacc.Bacc(None, target_bir_lowering=False, debug=False)

    xT = nc.dram_tensor("xT", [DM, s_len], MM_DT, kind="ExternalInput").ap()
    wqkT = nc.dram_tensor("wqkT", [DM, 1024], MM_DT, kind="ExternalInput").ap()
    wvT = nc.dram_tensor("wvT", [DM, 512], MM_DT, kind="ExternalInput").ap()
    woT = nc.dram_tensor("woT", [512, DM], MM_DT, kind="ExternalInput").ap()
    cosA = nc.dram_tensor("cosA", [128, s_len], F32, kind="ExternalInput").ap()
    sinA = nc.dram_tensor("sinA", [128, s_len], F32, kind="ExternalInput").ap()
    maskH = nc.dram_tensor("maskH", [128, 2048], F32, kind="ExternalInput").ap()
    ones8 = nc.dram_tensor("ones8", [128, 8], MM_DT, kind="ExternalInput").ap()
    onesb = nc.dram_tensor("onesb", [1, 64], MM_DT, kind="ExternalInput").ap()
    pat2 = nc.dram_tensor("pat2", [2, 128], MM_DT, kind="ExternalInput").ap()
    y = nc.dram_tensor("y", [s_len, DM], F32, kind="ExternalOutput").ap()

    nsc = s_len // SC  # number of 512-wide s-chunks
    TD = MM_DT         # dtype of matmul-feeding tiles

    def f(ap):
        # read view for DVE/ACT ops on matmul-feeding (f32r) tiles
        return ap.bitcast(F32)

    with tile.TileContext(nc) as tc:
        with ExitStack() as ctx:
            # ---- persistent pools (whole kernel) ----
            qk_pool = ctx.enter_context(tc.tile_pool(name="qk", bufs=1))
            va_pool = ctx.enter_context(tc.tile_pool(name="va", bufs=1))

            qkT = [qk_pool.tile([128, s_len], TD, tag=f"qkT{t}", name=f"qkT{t}")
                   for t in range(8)]
            v_aug = [va_pool.tile([128, 8 * 65], TD, tag=f"va{t}", name=f"va{t}")
                     for t in range(4 * nsc)]

            # ================= Phase 1: projections + RoPE =================
            with ExitStack() as pctx:
                proj_ps = pctx.enter_context(
                    tc.tile_pool(name="proj_ps", bufs=8, space="PSUM"))
                cpool = pctx.enter_context(tc.tile_pool(name="cst", bufs=1))
                xt_pool = pctx.enter_context(tc.tile_pool(name="xt", bufs=12))
                wqk_pool = pctx.enter_context(tc.tile_pool(name="wqk", bufs=16))
                wv_pool = pctx.enter_context(tc.tile_pool(name="wv", bufs=1))
                sh_pool = pctx.enter_context(tc.tile_pool(name="sh", bufs=3))

                cosT = cpool.tile([128, s_len], F32, tag="cos")
                sinT = cpool.tile([128, s_len], F32, tag="sin")
                wv_t = [wv_pool.tile([128, 512], TD, tag=f"wv{kc}", name=f"wv{kc}")
                        for kc in range(KCN)]

                def load_cos_sin():
                    nc.gpsimd.dma_start(cosT[:], cosA[:])
                    nc.gpsimd.dma_start(sinT[:], sinA[:])

                def load_wv():
                    for kc in range(KCN):
                        nc.gpsimd.dma_start(wv_t[kc][:],
                                            wvT[128 * kc:128 * (kc + 1), :])

                def load_vones():
                    # ones columns of v_aug (disjoint from the value copies)
                    for vt in range(4 * nsc):
                        v3 = v_aug[vt][:].rearrange("p (h c) -> p h c", c=65)
                        nc.gpsimd.dma_start(
                            v3[:, :, 64:65],
                            ones8[:].rearrange("p (h c) -> p h c", c=1))

                for sc in range(nsc):
                    ssl = slice(SC * sc, SC * (sc + 1))
                    xt = []
                    wq0 = []
                    for kc in range(KCN):
                        t = xt_pool.tile([128, SC], TD, tag="xt", name="xt")
                        nc.sync.dma_start(t[:], xT[128 * kc:128 * (kc + 1), ssl])
                        xt.append(t[:])
                        w = wqk_pool.tile([128, 512], TD, tag="wqk", name="wqk")
                        nc.sync.dma_start(w[:], wqkT[128 * kc:128 * (kc + 1),
                                                     0:512])
                        wq0.append(w)

                    # q (half=0) and k (half=1) projections -> qkT tiles
                    for half in range(2):
                        if half == 0:
                            wq = [w[:] for w in wq0]
                        else:
                            wq = []
                            for kc in range(KCN):
                                t = wqk_pool.tile([128, 512], TD, tag="wqk",
                                                  name="wqk")
                                nc.sync.dma_start(
                                    t[:], wqkT[128 * kc:128 * (kc + 1),
                                               512 * half:512 * (half + 1)])
                                wq.append(t[:])
                        if sc == 0:
                            load_cos_sin() if half == 0 else load_wv()
                        for mm in range(4):
                            mg = 4 * half + mm
                            ps = proj_ps.tile([128, SC], F32, tag="pj", name="psa")
                            for kc in range(KCN):
                                nc.tensor.matmul(
                                    ps[:], wq[kc][:, 128 * mm:128 * (mm + 1)],
                                    xt[kc],
                                    start=(kc == 0), stop=(kc == KCN - 1))
                            # RoPE fold: qkT = ps*cos + pairswap(ps)*sin
                            shuf = sh_pool.tile([128, SC], F32, tag="sh", name="shuf")
                            nc.vector.stream_shuffle(shuf[:], ps[:], PAIRSWAP)
                            nc.vector.tensor_mul(qkT[mg][:, ssl], ps[:], cosT[:, ssl])
                            nc.gpsimd.tensor_mul(shuf[:], shuf[:], sinT[:, ssl])
                            nc.vector.tensor_add(qkT[mg][:, ssl],
                                                 f(qkT[mg][:, ssl]), shuf[:])

                    # v projection -> v_aug tiles (natural [s, d] layout)
                    for sv in range(4):
                        ps = proj_ps.tile([128, SC], F32, tag="pj", name="psa")
                        for kc in range(KCN):
                            nc.tensor.matmul(
                                ps[:], xt[kc][:, 128 * sv:128 * (sv + 1)],
                                wv_t[kc][:],
                                start=(kc == 0), stop=(kc == KCN - 1))
                        vt = 4 * sc + sv
                        v3 = v_aug[vt][:].rearrange("p (h c) -> p h c", c=65)
                        nc.scalar.copy(
                            v3[:, :, 0:64],
                            ps[:].rearrange("p (h c) -> p h c", c=64))
                    if sc == 0:
                        load_vones()

            # ================= Phase 2: attention + out_proj ===============
            with ExitStack() as actx:
                ps_acc = actx.enter_context(
                    tc.tile_pool(name="ps_acc", bufs=3, space="PSUM"))
                ps_out = actx.enter_context(
                    tc.tile_pool(name="ps_out", bufs=2, space="PSUM"))
                apool = actx.enter_context(tc.tile_pool(name="att", bufs=1))
                p_pool = actx.enter_context(tc.tile_pool(name="pp", bufs=5))
                oc_pool = actx.enter_context(tc.tile_pool(name="oc", bufs=8))
                ocu_pool = actx.enter_context(tc.tile_pool(name="ocu", bufs=10))
                rc_pool = actx.enter_context(tc.tile_pool(name="rc", bufs=2))
                wo_pool = actx.enter_context(tc.tile_pool(name="wo", bufs=1))
                y_pool = actx.enter_context(tc.tile_pool(name="yst", bufs=3))

                maskT = apool.tile([128, 2048], F32, tag="mask", name="maskT")
                nc.sync.dma_start(maskT[:], maskH[:])
                onesT = apool.tile([65, 64], TD, tag="ones", name="onesT")
                nc.sync.dma_start(onesT[64:65, :], onesb[:])
                patT = apool.tile([2, 128], TD, tag="pat", name="patT")
                nc.sync.dma_start(patT[:], pat2[:])
                wo_t = [wo_pool.tile([128, DM], TD, tag=f"wo{k}", name=f"wo{k}")
                        for k in range(4)]
                for k in range(4):
                    nc.sync.dma_start(wo_t[k][:], woT[128 * k:128 * (k + 1), :])

                def attention_qc(qc, ocU, fillers):
                    """All 4 head pairs of q-chunk qc as one flattened
                    (pair, kb) stream with scores emitted 2 steps ahead
                    ACROSS pair boundaries, deferred out_proj chunks dripped
                    in as PE filler, and per-pair evacuation of unnormalized
                    out^T (row 64 = raw denominator) into ocU."""
                    nblk = 4 * qc + 4
                    outT = {}
                    sc_ps = {}

                    def q0_of(kb):
                        j = kb - 4 * qc
                        return 128 * j if j >= 0 else 0

                    def emit_scores(p, kb):
                        qT, kT = qkT[p], qkT[4 + p]
                        q0 = q0_of(kb)
                        ksl = slice(128 * kb, 128 * (kb + 1))
                        ps = ps_acc.tile([128, 1024], F32, tag="psA", name="scps")
                        nc.tensor.matmul(
                            ps[:, q0:512],
                            kT[0:64, ksl],
                            qT[0:64, SC * qc + q0:SC * (qc + 1)],
                            start=True, stop=True, tile_position=(0, 0))
                        nc.tensor.matmul(
                            ps[:, 512 + q0:1024],
                            kT[64:128, ksl],
                            qT[64:128, SC * qc + q0:SC * (qc + 1)],
                            start=True, stop=True, tile_position=(64, 0))
                        sc_ps[p, kb] = ps

                    def emit_softmax_pv(p, kb):
                        q0 = q0_of(kb)
                        j = kb - 4 * qc
                        ps = sc_ps.pop((p, kb))
                        if kb == 0:
                            outT[p, 0] = ps_out.tile([65, SC], F32,
                                                     tag="ps_out", name="outA")
                            outT[p, 1] = ps_out.tile([65, SC], F32,
                                                     tag="ps_out", name="outB")
                        P = p_pool.tile([128, 1024], TD, tag="P", name="Pt")
                        vps = ps[:].rearrange("p (two q) -> p two q", two=2)
                        vP = P[:].rearrange("p (two q) -> p two q", two=2)
                        nc.scalar.activation(vP[:, :, q0:512], vps[:, :, q0:512],
                                             AF.Exp, scale=SCALE)
                        if j >= 0:
                            msl = slice(512 * j + q0, 512 * (j + 1))
                            nc.vector.tensor_mul(P[:, q0:512], f(P[:, q0:512]),
                                                 maskT[:, msl])
                            nc.vector.tensor_mul(P[:, 512 + q0:1024],
                                                 f(P[:, 512 + q0:1024]),
                                                 maskT[:, msl])
                        va = v_aug[kb]
                        nc.tensor.matmul(
                            outT[p, 0][:, q0:512], va[:, 130 * p:130 * p + 65],
                            P[:, q0:512],
                            start=(kb == 0), stop=(kb == nblk - 1))
                        nc.tensor.matmul(
                            outT[p, 1][:, q0:512],
                            va[:, 130 * p + 65:130 * p + 130],
                            P[:, 512 + q0:1024],
                            start=(kb == 0), stop=(kb == nblk - 1))

                    stream = [(p, kb) for p in range(4) for kb in range(nblk)]
                    emitted = 0
                    for idx, (p, kb) in enumerate(stream):
                        while emitted <= idx + 2 and emitted < len(stream):
                            emit_scores(*stream[emitted])
                            emitted += 1
                        emit_softmax_pv(p, kb)
                        if fillers and idx % 3 == 2:
                            fillers.pop(0)()
                        if kb == nblk - 1:
                            for half in (0, 1):
                                u = ocu_pool.tile([65, SC], TD, tag="ocu",
                                                  name="ocu")
                                nc.vector.tensor_copy(u[:],
                                                      outT.pop((p, half))[:])
                                ocU[2 * p + half] = u

                def normalize(qc, ocU):
                    """Broadcast raw denominators via PE, reciprocal via
                    ACT exp(-ln(x)) (same table set as the softmax exp),
                    then the normalize muls. Returns the oc tiles."""
                    oc_t = [oc_pool.tile([128, SC], TD, tag="oc", name="oc")
                            for _ in range(4)]
                    for p in range(4):
                        # stage the two denominator rows at partitions 0/1
                        # (DMA has no partition-offset restrictions), then one
                        # K=2 selector matmul broadcasts both heads at once.
                        dn2 = rc_pool.tile([2, SC], TD, tag="dn2", name="dn2")
                        nc.sync.dma_start(dn2[0:1, :], ocU[2 * p][64:65, :])
                        nc.sync.dma_start(dn2[1:2, :], ocU[2 * p + 1][64:65, :])
                        bcq = ps_acc.tile([128, 1024], F32, tag="psA", name="bcq")
                        nc.tensor.matmul(bcq[:, 0:SC], patT[:], dn2[:],
                                         start=True, stop=True)
                        lnT = rc_pool.tile([128, SC], F32, tag="lnT", name="lnT")
                        nc.scalar.activation(lnT[:], bcq[:, 0:SC], AF.Ln)
                        nc.scalar.activation(bcq[:, SC:2 * SC], lnT[:],
                                             AF.Exp, scale=-1.0)
                        nc.vector.tensor_mul(oc_t[p][0:64, :],
                                             f(ocU[2 * p][0:64, :]),
                                             bcq[0:64, SC:2 * SC])
                        nc.vector.tensor_mul(oc_t[p][64:128, :],
                                             f(ocU[2 * p + 1][0:64, :]),
                                             bcq[64:128, SC:2 * SC])
                    return oc_t

                def outproj_chunk(qc, oc_t, sv):
                    """One s-row block of out_proj: both 512-wide n-halves
                    share a psum slot and each loaded weight serves two
                    matmuls; one evac + one store."""
                    svsl = slice(128 * sv, 128 * (sv + 1))
                    ps = ps_acc.tile([128, 1024], F32, tag="psA", name="psy")
                    for k in range(4):
                        nc.tensor.matmul(ps[:, 0:512], oc_t[k][:, svsl],
                                         wo_t[k][:, 0:512],
                                         start=(k == 0), stop=(k == 3))
                        nc.tensor.matmul(ps[:, 512:1024], oc_t[k][:, svsl],
                                         wo_t[k][:, 512:1024],
                                         start=(k == 0), stop=(k == 3))
                    yt = y_pool.tile([128, 1024], F32, tag="yst", name="yt")
                    nc.vector.tensor_copy(yt[:], ps[:])
                    nc.sync.dma_start(
                        y[SC * qc + 128 * sv:SC * qc + 128 * (sv + 1), :],
                        yt[:])

                pending = None
                fillers = []
                for qc in range(nsc):
                    ocU = [None] * 8
                    if pending is not None:
                        pqc, pocU = pending
                        oc_t = normalize(pqc, pocU)
                        fillers.extend(
                            (lambda sv=sv, q=pqc, o=oc_t:
                             outproj_chunk(q, o, sv)) for sv in range(4))
                        pending = None
                    attention_qc(qc, ocU, fillers)
                    pending = (qc, ocU)
                for fn in fillers:
                    fn()
                pqc, pocU = pending
                oc_t = normalize(pqc, pocU)
                for sv in range(4):
                    outproj_chunk(pqc, oc_t, sv)

    nc.compile()
    return nc


# ---------------------------------------------------------------------------
# Host-side input preparation
# ---------------------------------------------------------------------------

def _rope_tables(s_len):
    perm = np.empty(64, dtype=np.int64)
    perm[0::2] = np.arange(32)
    perm[1::2] = np.arange(32) + 32
    inv_freq = 1.0 / (ROPE_BASE ** (np.arange(0, D, 2, dtype=np.float32) / D))
    t = np.arange(s_len, dtype=np.float32)
    freqs = np.einsum('i,j->ij', t, inv_freq)           # [S, 32]
    emb = np.concatenate([freqs, freqs], axis=-1)       # [S, 64]
    cos = np.cos(emb).T.astype(np.float32)              # [64, S]
    sin = np.sin(emb).T.astype(np.float32)
    cos64 = cos[perm]
    sin64 = sin[perm]
    sign = np.where(perm < 32, -1.0, 1.0).astype(np.float32)[:, None]
    sin64 = sin64 * sign
    cosA = np.ascontiguousarray(np.tile(cos64, (2, 1)))
    sinA = np.ascontiguousarray(np.tile(sin64, (2, 1)))
    return perm, cosA, sinA


def _mask_tiles():
    k = np.arange(128)[:, None]
    q = np.arange(512)[None, :]
    blocks = [(128 * j + k <= q).astype(np.float32) for j in range(4)]
    return np.ascontiguousarray(np.concatenate(blocks, axis=1))  # [128, 2048]


def make_in_maps(x, W_qkv, W_out, s_len=S_FULL):
    B = x.shape[0]
    perm, cosA, sinA = _rope_tables(s_len)
    maskH = _mask_tiles()
    in_maps = []
    for c in range(2 * B):
        b, hg = c // 2, c % 2
        xTb = np.ascontiguousarray(x[b, :s_len].T.astype(np.float32))
        cols = []
        for h in range(HPC):
            cols.append(W_qkv[64 * (HPC * hg + h) + perm])          # q head
        for h in range(HPC):
            cols.append(W_qkv[1024 + 64 * (HPC * hg + h) + perm])   # k head
        wqkT = np.ascontiguousarray(np.concatenate(cols, axis=0).T)
        wvT = np.ascontiguousarray(
            W_qkv[2048 + 512 * hg:2048 + 512 * (hg + 1)].T)
        woT = np.ascontiguousarray(W_out[:, 512 * hg:512 * (hg + 1)].T)
        in_maps.append({
            "xT": xTb, "wqkT": wqkT, "wvT": wvT, "woT": woT,
            "cosA": cosA, "sinA": sinA, "maskH": maskH,
            "ones8": np.ones((128, 8), dtype=np.float32),
            "onesb": np.ones((1, 64), dtype=np.float32),
            "pat2": np.concatenate([
                np.concatenate([np.ones((1, 64)), np.zeros((1, 64))], axis=1),
                np.concatenate([np.zeros((1, 64)), np.ones((1, 64))], axis=1),
            ], axis=0).astype(np.float32),
        })
    return in_maps


_NC_CACHE = {}


def _get_program(s_len=S_FULL):
    if s_len not in _NC_CACHE:
        _NC_CACHE[s_len] = build_program(s_len)
    return _NC_CACHE[s_len]


def kernel(x, W_qkv, W_out):
    """Full-input, full-output causal self-attention on 8 NeuronCores."""
    _install_ntff_hook_shim()
    x = np.asarray(x, dtype=np.float32)
    W_qkv = np.asarray(W_qkv, dtype=np.float32)
    W_out = np.asarray(W_out, dtype=np.float32)
    B, S, dm = x.shape

    nc = _get_program(S)
    in_maps = make_in_maps(x, W_qkv, W_out, S)
    res = bass_utils.run_bass_kernel_spmd(nc, in_maps, list(range(2 * B)))
    out = np.empty((B, S, dm), dtype=np.float32)
    for b in range(B):
        out[b] = res.results[2 * b]["y"] + res.results[2 * b + 1]["y"]
    return out
